# revision 5
# baseline (speedup 1.0000x reference)
"""Trainium2 Bass kernel for CosimLoss.

loss = max(0, margin - mean(cos[label==1]) + mean(cos[label==-1]))
where cos[i] = <q_i, a_i> / (max(||q_i||, eps) * max(||a_i||, eps)).

Sharding: data-parallel over the batch axis across 8 NeuronCores
(1024 rows each). Each core computes per-row dot products and squared
norms with one fused DVE multiply-reduce plus two ACT square-accumulate
passes per [128, 4096] tile (DMA-bound, ~32 MiB/core), finishes the
cosine math on-chip, and emits per-partition masked partial sums
[128, 3] = (sum cos*mask_true, sum cos, count_true). The host sums the
8*128 partials and applies the hinge.
"""

import sys

import numpy as np

B, D = 8192, 4096
N_CORES = 8
B_LOC = B // N_CORES  # 1024 rows per core
P = 128               # SBUF partitions
T = B_LOC // P        # 8 row-tiles per core
EPS = 1e-8            # torch cosine_similarity norm clamp

_CACHE = {}


def _import_concourse():
    try:
        import concourse.bass  # noqa: F401
    except ImportError:
        sys.path.insert(0, "/opt/trn_rl_repo")
    import concourse.bass as bass
    import concourse.tile as tile
    from concourse import mybir
    from concourse.bass_utils import run_bass_kernel_spmd

    return bass, tile, mybir, run_bass_kernel_spmd


def _split_multi_waits(nc, mybir):
    """The walrus build in this container encodes at most ONE sem-wait per
    instruction ("Too many sync wait commands"). Tile attaches every
    outstanding semaphore wait to a single instruction (e.g. the tail
    drain waits on all DMA lanes). Rewrite: hoist all but the last wait
    onto single-wait NOPs on the same engine, inserted right before the
    instruction (same-engine program order preserves semantics)."""
    for fn in nc.m.functions:
        for blk in fn.blocks:
            lst = blk.instructions
            i = 0
            while i < len(lst):
                inst = lst[i]
                si = inst.sync_info
                if si is not None and len(si.on_wait) > 1:
                    waits = list(si.on_wait)
                    for j, w in enumerate(waits[:-1]):
                        nop = mybir.InstNoOp(name=f"{inst.name}-wsplit{j}", ins=[], outs=[])
                        nop.engine = inst.engine
                        nop.sync_info = mybir.SyncInfo(on_wait=[w], on_update=[])
                        lst.insert(i, nop)
                        i += 1
                    inst.sync_info = mybir.SyncInfo(
                        on_wait=[waits[-1]], on_update=list(si.on_update)
                    )
                i += 1


def _build(bass, tile, mybir):
    f32 = mybir.dt.float32
    Alu = mybir.AluOpType
    Act = mybir.ActivationFunctionType

    nc = bass.Bass("TRN2", target_bir_lowering=False, debug=False)
    q = nc.declare_dram_parameter("q", [B_LOC, D], f32, isOutput=False)
    a = nc.declare_dram_parameter("a", [B_LOC, D], f32, isOutput=False)
    lab = nc.declare_dram_parameter("lab", [B_LOC, 1], mybir.dt.int32, isOutput=False)
    out = nc.declare_dram_parameter("out", [P, 3], f32, isOutput=True)

    with tile.TileContext(nc) as tc:
        with (
            tc.tile_pool(name="io", bufs=4) as io,
            tc.tile_pool(name="small", bufs=1) as small,
        ):
            # Labels for row r = t*P + p land at [p, t] to match the cos layout.
            lab_i = small.tile([P, T], mybir.dt.int32)
            nc.sync.dma_start(out=lab_i, in_=lab.ap().rearrange("(t p) o -> p (t o)", p=P))

            dot_all = small.tile([P, T], f32)
            qq_all = small.tile([P, T], f32)
            aa_all = small.tile([P, T], f32)

            for t in range(T):
                qt = io.tile([P, D], f32, tag="qt")
                at = io.tile([P, D], f32, tag="at")
                nc.sync.dma_start(out=qt, in_=q.ap()[t * P:(t + 1) * P, :])
                nc.sync.dma_start(out=at, in_=a.ap()[t * P:(t + 1) * P, :])
                # Row dot products on DVE: prod then free-dim reduce.
                # (tensor_tensor_reduce mis-encodes for this walrus build.)
                prod = io.tile([P, D], f32, tag="prod", bufs=2)
                nc.vector.tensor_mul(prod, qt, at)
                nc.vector.reduce_sum(
                    dot_all[:, t:t + 1], prod, axis=mybir.AxisListType.X
                )
                # Row squared norms on ACT (in-place elementwise out).
                nc.scalar.activation(qt, qt, Act.Square, accum_out=qq_all[:, t:t + 1])
                nc.scalar.activation(at, at, Act.Square, accum_out=aa_all[:, t:t + 1])

            # cos = dot / (max(sqrt(qq), eps) * max(sqrt(aa), eps))
            nc.scalar.sqrt(qq_all, qq_all)
            nc.scalar.sqrt(aa_all, aa_all)
            nc.vector.tensor_scalar_max(qq_all, qq_all, EPS)
            nc.vector.tensor_scalar_max(aa_all, aa_all, EPS)
            den = small.tile([P, T], f32)
            nc.vector.tensor_mul(den, qq_all, aa_all)
            rden = small.tile([P, T], f32)
            nc.vector.reciprocal(rden, den)
            cos = small.tile([P, T], f32)
            nc.vector.tensor_mul(cos, dot_all, rden)

            # w = (label + 1)/2 in {0,1}; res = [s_true, s_all, n_true]
            labf = small.tile([P, T], f32)
            nc.vector.tensor_copy(labf, lab_i)
            w = small.tile([P, T], f32)
            res = small.tile([P, 3], f32)
            nc.vector.tensor_scalar(
                w, labf, 1.0, 0.5, op0=Alu.add, op1=Alu.mult,
                accum_out=res[:, 2:3],
            )
            wcos = small.tile([P, T], f32)
            nc.vector.tensor_mul(wcos, cos, w)
            nc.vector.reduce_sum(res[:, 0:1], wcos, axis=mybir.AxisListType.X)
            nc.vector.reduce_sum(res[:, 1:2], cos, axis=mybir.AxisListType.X)
            nc.sync.dma_start(out=out.ap(), in_=res)

    _split_multi_waits(nc, mybir)
    return nc


def _get_nc():
    if "nc" not in _CACHE:
        bass, tile, mybir, run_spmd = _import_concourse()
        _CACHE["run_spmd"] = run_spmd
        _CACHE["nc"] = _build(bass, tile, mybir)
    return _CACHE["nc"], _CACHE["run_spmd"]


def run_device(ques, ans, label, **spmd_kwargs):
    """Run the 8-core SPMD kernel; returns (partials [8,128,3], BassKernelResults)."""
    nc, run_spmd = _get_nc()
    ques = np.ascontiguousarray(np.asarray(ques, dtype=np.float32))
    ans = np.ascontiguousarray(np.asarray(ans, dtype=np.float32))
    label = np.ascontiguousarray(np.asarray(label, dtype=np.int32))
    in_maps = [
        {
            "q": ques[c * B_LOC:(c + 1) * B_LOC],
            "a": ans[c * B_LOC:(c + 1) * B_LOC],
            "lab": label[c * B_LOC:(c + 1) * B_LOC],
        }
        for c in range(N_CORES)
    ]
    br = run_spmd(nc, in_maps, list(range(N_CORES)), **spmd_kwargs)
    partials = np.stack([br.results[c]["out"] for c in range(N_CORES)])
    return partials, br


def finish(partials, margin):
    s = partials.reshape(-1, 3).astype(np.float64).sum(axis=0)
    s_true, s_all, n_true = s
    n_false = float(B) - n_true
    true_ave = s_true / n_true
    false_ave = (s_all - s_true) / n_false
    m = float(np.asarray(margin).reshape(-1)[0])
    loss = max(0.0, m - true_ave + false_ave)
    return np.array([loss], dtype=np.float32)


def kernel(ques, ans, label, margin):
    partials, _ = run_device(ques, ans, label)
    return finish(partials, margin)


# revision 6
# speedup vs baseline: 49366.4620x; 49366.4620x over previous
"""Trainium2 Bass kernel for CosimLoss.

loss = max(0, margin - mean(cos[label==1]) + mean(cos[label==-1]))
where cos[i] = <q_i, a_i> / (max(||q_i||, eps) * max(||a_i||, eps)).

Sharding: data-parallel over the batch axis across 8 NeuronCores
(1024 rows each). Each core computes per-row dot products and squared
norms with one fused DVE multiply-reduce plus two ACT square-accumulate
passes per [128, 4096] tile (DMA-bound, ~32 MiB/core), finishes the
cosine math on-chip, and emits per-partition masked partial sums
[128, 3] = (sum cos*mask_true, sum cos, count_true). The host sums the
8*128 partials and applies the hinge.
"""

import sys

import numpy as np

B, D = 8192, 4096
N_CORES = 8
B_LOC = B // N_CORES  # 1024 rows per core
P = 128               # SBUF partitions
T = B_LOC // P        # 8 row-tiles per core
EPS = 1e-8            # torch cosine_similarity norm clamp

_CACHE = {}


def _import_concourse():
    try:
        import concourse.bass  # noqa: F401
    except ImportError:
        sys.path.insert(0, "/opt/trn_rl_repo")
    import concourse.bass as bass
    import concourse.tile as tile
    from concourse import mybir
    from concourse.bass_utils import run_bass_kernel_spmd

    return bass, tile, mybir, run_bass_kernel_spmd


def _split_multi_waits(nc, mybir):
    """The walrus build in this container encodes at most ONE sem-wait per
    instruction ("Too many sync wait commands"). Tile attaches every
    outstanding semaphore wait to a single instruction (e.g. the tail
    drain waits on all DMA lanes). Rewrite: hoist all but the last wait
    onto single-wait NOPs on the same engine, inserted right before the
    instruction (same-engine program order preserves semantics)."""
    for fn in nc.m.functions:
        for blk in fn.blocks:
            lst = blk.instructions
            i = 0
            while i < len(lst):
                inst = lst[i]
                si = inst.sync_info
                if si is not None and len(si.on_wait) > 1:
                    waits = list(si.on_wait)
                    for j, w in enumerate(waits[:-1]):
                        nop = mybir.InstNoOp(name=f"{inst.name}-wsplit{j}", ins=[], outs=[])
                        nop.engine = inst.engine
                        nop.sync_info = mybir.SyncInfo(on_wait=[w], on_update=[])
                        lst.insert(i, nop)
                        i += 1
                    inst.sync_info = mybir.SyncInfo(
                        on_wait=[waits[-1]], on_update=list(si.on_update)
                    )
                i += 1


def _build(bass, tile, mybir):
    f32 = mybir.dt.float32
    Alu = mybir.AluOpType
    Act = mybir.ActivationFunctionType

    nc = bass.Bass("TRN2", target_bir_lowering=False, debug=False)
    q = nc.declare_dram_parameter("q", [B_LOC, D], f32, isOutput=False)
    a = nc.declare_dram_parameter("a", [B_LOC, D], f32, isOutput=False)
    lab = nc.declare_dram_parameter("lab", [B_LOC, 1], mybir.dt.int32, isOutput=False)
    out = nc.declare_dram_parameter("out", [P, 3], f32, isOutput=True)

    with tile.TileContext(nc) as tc:
        with (
            tc.tile_pool(name="io", bufs=4) as io,
            tc.tile_pool(name="small", bufs=1) as small,
        ):
            # Labels for row r = t*P + p land at [p, t] to match the cos layout.
            lab_i = small.tile([P, T], mybir.dt.int32)
            nc.sync.dma_start(out=lab_i, in_=lab.ap().rearrange("(t p) o -> p (t o)", p=P))

            dot_all = small.tile([P, T], f32)
            qq_all = small.tile([P, T], f32)
            aa_all = small.tile([P, T], f32)

            for t in range(T):
                qt = io.tile([P, D], f32, tag="qt")
                at = io.tile([P, D], f32, tag="at")
                nc.sync.dma_start(out=qt, in_=q.ap()[t * P:(t + 1) * P, :])
                nc.sync.dma_start(out=at, in_=a.ap()[t * P:(t + 1) * P, :])
                # Row dot products on DVE: prod then free-dim reduce.
                # (tensor_tensor_reduce mis-encodes for this walrus build.)
                prod = io.tile([P, D], f32, tag="prod", bufs=2)
                nc.vector.tensor_mul(prod, qt, at)
                nc.vector.reduce_sum(
                    dot_all[:, t:t + 1], prod, axis=mybir.AxisListType.X
                )
                # Row squared norms on ACT (in-place elementwise out).
                nc.scalar.activation(qt, qt, Act.Square, accum_out=qq_all[:, t:t + 1])
                nc.scalar.activation(at, at, Act.Square, accum_out=aa_all[:, t:t + 1])

            # cos = dot / (max(sqrt(qq), eps) * max(sqrt(aa), eps))
            nc.scalar.sqrt(qq_all, qq_all)
            nc.scalar.sqrt(aa_all, aa_all)
            nc.vector.tensor_scalar_max(qq_all, qq_all, EPS)
            nc.vector.tensor_scalar_max(aa_all, aa_all, EPS)
            den = small.tile([P, T], f32)
            nc.vector.tensor_mul(den, qq_all, aa_all)
            rden = small.tile([P, T], f32)
            nc.vector.reciprocal(rden, den)
            cos = small.tile([P, T], f32)
            nc.vector.tensor_mul(cos, dot_all, rden)

            # w = (label + 1)/2 in {0,1}; res = [s_true, s_all, n_true]
            labf = small.tile([P, T], f32)
            nc.vector.tensor_copy(labf, lab_i)
            w = small.tile([P, T], f32)
            res = small.tile([P, 3], f32)
            # (two single-scalar ops: this walrus drops tensor_scalar's
            # second op and corrupts its accum_out)
            nc.vector.tensor_scalar_add(w, labf, 1.0)
            nc.vector.tensor_scalar_mul(w, w, 0.5)
            nc.vector.reduce_sum(res[:, 2:3], w, axis=mybir.AxisListType.X)
            wcos = small.tile([P, T], f32)
            nc.vector.tensor_mul(wcos, cos, w)
            nc.vector.reduce_sum(res[:, 0:1], wcos, axis=mybir.AxisListType.X)
            nc.vector.reduce_sum(res[:, 1:2], cos, axis=mybir.AxisListType.X)
            nc.sync.dma_start(out=out.ap(), in_=res)

    _split_multi_waits(nc, mybir)
    return nc


def _get_nc():
    if "nc" not in _CACHE:
        bass, tile, mybir, run_spmd = _import_concourse()
        _CACHE["run_spmd"] = run_spmd
        _CACHE["nc"] = _build(bass, tile, mybir)
    return _CACHE["nc"], _CACHE["run_spmd"]


def run_device(ques, ans, label, **spmd_kwargs):
    """Run the 8-core SPMD kernel; returns (partials [8,128,3], BassKernelResults)."""
    nc, run_spmd = _get_nc()
    ques = np.ascontiguousarray(np.asarray(ques, dtype=np.float32))
    ans = np.ascontiguousarray(np.asarray(ans, dtype=np.float32))
    label = np.ascontiguousarray(np.asarray(label, dtype=np.int32))
    in_maps = [
        {
            "q": ques[c * B_LOC:(c + 1) * B_LOC],
            "a": ans[c * B_LOC:(c + 1) * B_LOC],
            "lab": label[c * B_LOC:(c + 1) * B_LOC],
        }
        for c in range(N_CORES)
    ]
    br = run_spmd(nc, in_maps, list(range(N_CORES)), **spmd_kwargs)
    partials = np.stack([br.results[c]["out"] for c in range(N_CORES)])
    return partials, br


def finish(partials, margin):
    s = partials.reshape(-1, 3).astype(np.float64).sum(axis=0)
    s_true, s_all, n_true = s
    n_false = float(B) - n_true
    true_ave = s_true / n_true
    false_ave = (s_all - s_true) / n_false
    m = float(np.asarray(margin).reshape(-1)[0])
    loss = max(0.0, m - true_ave + false_ave)
    return np.array([loss], dtype=np.float32)


def kernel(ques, ans, label, margin):
    partials, _ = run_device(ques, ans, label)
    return finish(partials, margin)
